# revision 2
# baseline (speedup 1.0000x reference)
"""Bilinear interpolation (affine grid sample) TRN2 Bass kernel (active: v2 zipper).

v2 vs baseline: row-pair "zipper" bf16 layout in DRAM so each output point
needs ONE dma_gather descriptor (768 B covering 3 px x 2 rows x 64 ch bf16)
instead of two 768-B f32 fetches. The kernel is descriptor-rate bound
(~33 ns/descriptor SWDGE), so halving descriptors halves runtime.

Zipper layout Z[y, x, r, c] bf16: entry (y, x) holds rows (y, y+1) of
column x, channels-last. Entry = 2*64 bf16 = 256 B. Gather window = 3
entries (stride 2 entries = 512 B), idx = y0*128 + (x0>>1) <= 32639 (int16).

Per batch (2 per core, 8 cores):
  A. Read X, PE-transpose [c, x] -> [x, c], cast f32->bf16, DMA to Z twice
     (row y data lands at Z[y,:,0,:] and Z[y-1,:,1,:]).
  B-D. theta scalars, wrapped gather indices, 6 combine weights (as baseline,
     weights cast to bf16).
  E. One dma_gather per chunk; 11-op DVE combine (bf16 in, f32 out via copy);
     PE transpose back [pt, c] -> [c, pt]; DMA out.
"""

import sys

sys.path.insert(0, "/opt/trn_rl_repo")

import numpy as np
import ml_dtypes  # noqa: F401

import concourse.bass as bass
import concourse.bacc as bacc
import concourse.mybir as mybir
from concourse import tile
from concourse.bass import AP
from concourse.masks import make_identity

F32 = mybir.dt.float32
BF16 = mybir.dt.bfloat16
I16 = mybir.dt.int16
I32 = mybir.dt.int32

B, C, H, W = 16, 64, 256, 256
HW = H * W
NCORES = 8
BPC = B // NCORES  # 2

NPTS = HW
NGRP = NPTS // 128  # 512
JCHUNK = 16
NCHUNK = NGRP // JCHUNK  # 32
CHUNK_PTS = JCHUNK * 128  # 2048

ENT = 2 * C  # zipper entry: 2 rows x 64 ch = 128 bf16 = 256 B
ELEM = 3 * ENT  # gather window: 3 entries = 384 bf16 = 768 B
ESTEP = 2 * ENT  # gather stride: 2 entries = 256 bf16 = 512 B
NPAIR = 32640  # max idx 32639 = 254*128 + 127

SCALE = 2.0 * (W / 2) / (W - 1)  # 256/255


def _host_consts():
    q = np.arange(128)
    s = np.arange(NPTS // 16)
    j = np.arange(NGRP)
    p = np.arange(128)
    ixw = 16.0 * (s % 16)[None, :] + (q % 16)[:, None]
    iyw = np.broadcast_to((s // 16)[None, :], (128, len(s))).astype(np.float64)
    pjx = 128.0 * (j % 2)[None, :] + p[:, None]
    pjy = np.broadcast_to((j // 2)[None, :], (128, NGRP)).astype(np.float64)
    return {
        "IXW": np.ascontiguousarray(ixw, "bfloat16"),
        "IYW": np.ascontiguousarray(iyw, "bfloat16"),
        "PJX": np.ascontiguousarray(pjx, "bfloat16"),
        "PJY": np.ascontiguousarray(pjy, "bfloat16"),
    }


def build_nc(n_batches=BPC, n_chunks=NCHUNK, debug=False):
    nc = bacc.Bacc("TRN2", target_bir_lowering=False, debug=debug)

    x_in = nc.declare_dram_parameter("X", [n_batches, C, HW], F32, isOutput=False)
    th_in = nc.declare_dram_parameter("THETA", [n_batches, 128, 6], F32, isOutput=False)
    ixw_in = nc.declare_dram_parameter("IXW", [128, NPTS // 16], BF16, isOutput=False)
    iyw_in = nc.declare_dram_parameter("IYW", [128, NPTS // 16], BF16, isOutput=False)
    pjx_in = nc.declare_dram_parameter("PJX", [128, NGRP], BF16, isOutput=False)
    pjy_in = nc.declare_dram_parameter("PJY", [128, NGRP], BF16, isOutput=False)
    out_ext = nc.declare_dram_parameter("OUT", [n_batches, C, HW], F32, isOutput=True)

    # zipper scratch: [y, x, r, c] bf16, 256 rows (row 255 r1 = spill pad)
    zip_t = nc.dram_tensor("ZIP", [H, W, 2, C], BF16)

    _regcache = {}

    with tile.TileContext(nc) as tc:
        import contextlib

        with contextlib.ExitStack() as ctx:
            cpool = ctx.enter_context(tc.tile_pool(name="consts", bufs=1))
            apool = ctx.enter_context(tc.tile_pool(name="hwcio", bufs=3))
            apsum = ctx.enter_context(tc.tile_pool(name="tpsum", bufs=2, space="PSUM"))
            wpool = ctx.enter_context(tc.tile_pool(name="weights", bufs=1))
            spool = ctx.enter_context(tc.tile_pool(name="scratch", bufs=1))
            gpool = ctx.enter_context(tc.tile_pool(name="gather", bufs=3))
            opool = ctx.enter_context(tc.tile_pool(name="outbuf", bufs=2))
            opsum = ctx.enter_context(tc.tile_pool(name="opsum", bufs=2, space="PSUM"))

            ident = cpool.tile([128, 128], F32)
            make_identity(nc, ident[:])
            ixw = cpool.tile([128, NPTS // 16], BF16)
            nc.sync.dma_start(out=ixw[:], in_=ixw_in.ap())
            iyw = cpool.tile([128, NPTS // 16], BF16)
            nc.sync.dma_start(out=iyw[:], in_=iyw_in.ap())
            pjx = cpool.tile([128, NGRP], BF16)
            nc.sync.dma_start(out=pjx[:], in_=pjx_in.ap())
            pjy = cpool.tile([128, NGRP], BF16)
            nc.sync.dma_start(out=pjy[:], in_=pjy_in.ap())

            # zero zipper row 255 (read only by zero-weight window spill)
            zt = cpool.tile([128, (W * 2 * C) // 128], BF16)
            nc.vector.memset(zt[:], 0.0)
            nc.sync.dma_start(
                out=zip_t.ap()[H - 1].rearrange("x r c -> (x r c)").rearrange(
                    "(p f) -> p f", p=128
                ),
                in_=zt[:],
            )

            V = nc.vector
            S = nc.scalar

            def tsc(out, in0, s1, op0, s2=None, op1=None):
                if s2 is None:
                    return V.tensor_scalar(out, in0, s1, None, op0)
                return V.tensor_scalar(out, in0, s1, s2, op0, op1)

            A = mybir.AluOpType

            for b in range(n_batches):
                xb = x_in.ap()[b]  # [64, HW]
                ob = out_ext.ap()[b]  # [64, HW]

                # ---------- Phase A: build zipper ----------
                # chunk = 8 rows (2048 px); 16 PE transposes -> 2 PSUM banks ->
                # bf16 stage [128 p, 8 y, 2 blk, 64 c] -> 2 DMAs (r0, r1-shift)
                for cc in range(32):
                    y0 = cc * 8
                    xin = apool.tile([C, 2048], F32, tag="xin")
                    nc.sync.dma_start(out=xin[:], in_=xb[:, y0 * W : (y0 + 8) * W])
                    stage = apool.tile([128, 8, 2, C], BF16, tag="stage")
                    for half in range(2):
                        ps = apsum.tile([128, 512], F32, tag="psA")
                        for t in range(8):
                            sl = half * 8 + t
                            nc.tensor.transpose(
                                ps[:, t * 64 : (t + 1) * 64],
                                xin[:, sl * 128 : (sl + 1) * 128],
                                ident[:C, :C],
                            )
                        S.copy(
                            stage[:, half * 4 : (half + 1) * 4, :, :],
                            ps[:].rearrange("p (y b c) -> p y b c", y=4, b=2),
                        )
                    # r0: rows y0..y0+7 -> Z[y0:y0+8, :, 0, :]
                    dst0 = zip_t.ap()[y0 : y0 + 8, :, 0, :].rearrange(
                        "y (blk p) c -> p y blk c", p=128
                    )
                    nc.sync.dma_start(out=dst0, in_=stage[:])
                    # r1: rows y0..y0+7 are row (y+1) data for Z[y0-1:y0+7, :, 1, :]
                    if y0 == 0:
                        dst1 = zip_t.ap()[0:7, :, 1, :].rearrange(
                            "y (blk p) c -> p y blk c", p=128
                        )
                        nc.sync.dma_start(out=dst1, in_=stage[:, 1:8])
                    else:
                        dst1 = zip_t.ap()[y0 - 1 : y0 + 7, :, 1, :].rearrange(
                            "y (blk p) c -> p y blk c", p=128
                        )
                        nc.sync.dma_start(out=dst1, in_=stage[:])

                # ---------- Phase B: theta scalars ----------
                thsb = spool.tile([128, 6], F32, tag="thsb")
                nc.sync.dma_start(out=thsb[:], in_=th_in.ap()[b])
                thb = thsb
                sc = spool.tile([128, 8], F32, tag="thsc")
                tsc(sc[:, 0:1], thb[:, 0:1], SCALE, A.mult)
                tsc(sc[:, 1:2], thb[:, 1:2], SCALE, A.mult)
                V.tensor_tensor(sc[:, 2:3], thb[:, 2:3], thb[:, 0:1], A.subtract)
                V.tensor_tensor(sc[:, 2:3], sc[:, 2:3], thb[:, 1:2], A.subtract)
                tsc(sc[:, 2:3], sc[:, 2:3], 1.0, A.add, float(W // 2), A.mult)
                tsc(sc[:, 3:4], thb[:, 3:4], SCALE, A.mult)
                tsc(sc[:, 4:5], thb[:, 4:5], SCALE, A.mult)
                V.tensor_tensor(sc[:, 5:6], thb[:, 5:6], thb[:, 3:4], A.subtract)
                V.tensor_tensor(sc[:, 5:6], sc[:, 5:6], thb[:, 4:5], A.subtract)
                tsc(sc[:, 5:6], sc[:, 5:6], 1.0, A.add, float(H // 2), A.mult)
                ax, bx, cx = sc[:, 0:1], sc[:, 1:2], sc[:, 2:3]
                ay, by, cy = sc[:, 3:4], sc[:, 4:5], sc[:, 5:6]

                # ---------- Phase C: wrapped gather indices ----------
                SW = NPTS // 16
                SH = SW // 2
                idx16 = wpool.tile([128, SW], I16)
                for hh in range(2):
                    hsl = slice(hh * SH, (hh + 1) * SH)
                    t0 = spool.tile([128, SH], F32, tag="wk0")
                    t1 = spool.tile([128, SH], F32, tag="wk1")
                    i0 = spool.tile([128, SH], I32, tag="wki0")
                    i1 = spool.tile([128, SH], I32, tag="wki1")
                    tsc(t0[:], ixw[:, hsl], ax, A.mult)
                    V.scalar_tensor_tensor(t0[:], iyw[:, hsl], bx, t0[:], A.mult, A.add)
                    tsc(t0[:], t0[:], cx, A.add, 0.0, A.max)
                    tsc(t0[:], t0[:], float(W - 2), A.min, 0.5, A.subtract)
                    V.tensor_copy(i0[:], t0[:])
                    tsc(i0[:], i0[:], 1, A.arith_shift_right)
                    tsc(t1[:], ixw[:, hsl], ay, A.mult)
                    V.scalar_tensor_tensor(t1[:], iyw[:, hsl], by, t1[:], A.mult, A.add)
                    tsc(t1[:], t1[:], cy, A.add, 0.0, A.max)
                    tsc(t1[:], t1[:], float(H - 2), A.min, 0.5, A.subtract)
                    V.tensor_copy(i1[:], t1[:])
                    tsc(i1[:], i1[:], 7, A.logical_shift_left)
                    V.tensor_tensor(i1[:], i1[:], i0[:], A.add)
                    V.tensor_copy(idx16[:, hsl], i1[:])

                # ---------- Phase D: weights [128, 512] x6, cast bf16 ----------
                Wt = wpool.tile([128, 6 * NGRP], BF16, tag="Wt")

                def wsl(i):
                    return Wt[:, i * NGRP : (i + 1) * NGRP]

                xW = spool.tile([128, NGRP], F32, tag="xW")
                yW = spool.tile([128, NGRP], F32, tag="yW")
                u0 = spool.tile([128, NGRP], F32, tag="u0")
                u1 = spool.tile([128, NGRP], F32, tag="u1")
                u2 = spool.tile([128, NGRP], F32, tag="u2")
                u3 = spool.tile([128, NGRP], F32, tag="u3")
                tsc(xW[:], pjx[:], ax, A.mult)
                V.scalar_tensor_tensor(xW[:], pjy[:], bx, xW[:], A.mult, A.add)
                tsc(xW[:], xW[:], cx, A.add)
                tsc(yW[:], pjx[:], ay, A.mult)
                V.scalar_tensor_tensor(yW[:], pjy[:], by, yW[:], A.mult, A.add)
                tsc(yW[:], yW[:], cy, A.add)
                tsc(u0[:], xW[:], 0.0, A.is_ge)
                tsc(u1[:], xW[:], float(W - 1), A.is_lt)
                V.tensor_tensor(u0[:], u0[:], u1[:], A.mult)
                tsc(u1[:], yW[:], 0.0, A.is_ge)
                V.tensor_tensor(u0[:], u0[:], u1[:], A.mult)
                tsc(u1[:], yW[:], float(H - 1), A.is_lt)
                V.tensor_tensor(u0[:], u0[:], u1[:], A.mult)
                iw0 = spool.tile([128, NGRP], I32, tag="iw0")
                tsc(u1[:], xW[:], 0.0, A.max)
                tsc(u2[:], u1[:], 0.5, A.subtract)
                V.tensor_copy(iw0[:], u2[:])
                V.tensor_copy(u2[:], iw0[:])
                V.tensor_tensor(u2[:], u1[:], u2[:], A.subtract)  # fx
                tsc(iw0[:], iw0[:], 1, A.bitwise_and)
                V.tensor_copy(u3[:], iw0[:])  # par
                tsc(u1[:], yW[:], 0.0, A.max)
                tsc(yW[:], u1[:], 0.5, A.subtract)
                V.tensor_copy(iw0[:], yW[:])
                V.tensor_copy(yW[:], iw0[:])
                V.tensor_tensor(yW[:], u1[:], yW[:], A.subtract)  # fy
                tsc(xW[:], yW[:], -1.0, A.mult, 1.0, A.add)
                V.tensor_tensor(xW[:], xW[:], u0[:], A.mult)  # wy0
                V.tensor_tensor(yW[:], yW[:], u0[:], A.mult)  # wy1
                parm = spool.tile([128, NGRP], F32, tag="parm")
                tsc(parm[:], u3[:], -1.0, A.mult, 1.0, A.add)
                wx0 = spool.tile([128, NGRP], F32, tag="wx0")
                tsc(wx0[:], u2[:], -1.0, A.mult, 1.0, A.add)
                V.tensor_tensor(u0[:], wx0[:], parm[:], A.mult)  # ws0
                V.tensor_tensor(u1[:], wx0[:], u3[:], A.mult)
                V.tensor_tensor(parm[:], u2[:], parm[:], A.mult)
                V.tensor_tensor(u1[:], u1[:], parm[:], A.add)  # ws1
                V.tensor_tensor(u2[:], u2[:], u3[:], A.mult)  # ws2
                # tap order within entry window: (x, r): w[x][r] = ws_x * wy_r
                V.tensor_tensor(wsl(0), xW[:], u0[:], A.mult)  # x0 r0
                V.tensor_tensor(wsl(1), yW[:], u0[:], A.mult)  # x0 r1
                V.tensor_tensor(wsl(2), xW[:], u1[:], A.mult)  # x1 r0
                V.tensor_tensor(wsl(3), yW[:], u1[:], A.mult)  # x1 r1
                V.tensor_tensor(wsl(4), xW[:], u2[:], A.mult)  # x2 r0
                V.tensor_tensor(wsl(5), yW[:], u2[:], A.mult)  # x2 r1

                # ---------- Phase E: gather + combine + transpose + out ----------
                zip_flat = zip_t.ap().flatten()
                in0 = AP(zip_flat.tensor, 0, [[ESTEP, NPAIR], [1, ELEM]])
                if "nreg" not in _regcache:
                    _regcache["nreg"] = nc.gpsimd.to_reg(CHUNK_PTS)
                nreg = _regcache["nreg"]
                for ci in range(n_chunks):
                    g0 = gpool.tile([128, JCHUNK, ELEM], BF16, tag="g0")
                    idxs = idx16[
                        :, ci * (CHUNK_PTS // 16) : (ci + 1) * (CHUNK_PTS // 16)
                    ]
                    nc.gpsimd.dma_gather(
                        g0[:], in0, idxs, CHUNK_PTS, nreg, ELEM,
                        elem_step=ESTEP, queue_num=0, single_packet=False,
                    )
                    comb = opool.tile([128, JCHUNK, C], F32, tag="comb")
                    tmp = opool.tile([128, JCHUNK, C], F32, tag="tmp")

                    def wv(i):
                        return (
                            wsl(i)[:, ci * JCHUNK : (ci + 1) * JCHUNK]
                            .unsqueeze(2)
                            .to_broadcast([128, JCHUNK, C])
                        )

                    def gs(x, r):
                        off = x * ENT + r * C
                        return g0[:, :, off : off + C]

                    V.tensor_tensor(comb[:], gs(0, 0), wv(0), A.mult)
                    V.tensor_tensor(tmp[:], gs(0, 1), wv(1), A.mult)
                    V.tensor_tensor(comb[:], comb[:], tmp[:], A.add)
                    V.tensor_tensor(tmp[:], gs(1, 0), wv(2), A.mult)
                    V.tensor_tensor(comb[:], comb[:], tmp[:], A.add)
                    V.tensor_tensor(tmp[:], gs(1, 1), wv(3), A.mult)
                    V.tensor_tensor(comb[:], comb[:], tmp[:], A.add)
                    V.tensor_tensor(tmp[:], gs(2, 0), wv(4), A.mult)
                    V.tensor_tensor(comb[:], comb[:], tmp[:], A.add)
                    V.tensor_tensor(tmp[:], gs(2, 1), wv(5), A.mult)
                    V.tensor_tensor(comb[:], comb[:], tmp[:], A.add)

                    outsb = opool.tile([C, JCHUNK * 128], F32, tag="outsb")
                    for q in range(JCHUNK // 4):
                        ps = opsum.tile([C, 512], F32, tag="psO")
                        for g in range(4):
                            gg = q * 4 + g
                            nc.tensor.transpose(
                                ps[:, g * 128 : (g + 1) * 128],
                                comb[:, gg, :],
                                ident[:],
                            )
                        S.copy(outsb[:, q * 512 : (q + 1) * 512], ps[:])
                    nc.sync.dma_start(
                        out=ob[:, ci * CHUNK_PTS : (ci + 1) * CHUNK_PTS],
                        in_=outsb[:],
                    )

    nc.compile()
    return nc


_CONSTS = _host_consts()


def _make_in_maps(X, theta, n_batches=BPC):
    Xr = np.ascontiguousarray(X.reshape(B, C, HW), np.float32)
    th = np.ascontiguousarray(theta, np.float32)
    in_maps = []
    for core in range(NCORES):
        b0 = core * n_batches
        th_rep = np.repeat(th[b0 : b0 + n_batches, None, :], 128, axis=1)
        in_maps.append(
            {
                "X": Xr[b0 : b0 + n_batches],
                "THETA": np.ascontiguousarray(th_rep, np.float32),
                **_CONSTS,
            }
        )
    return in_maps


_NC_CACHE = {}


def kernel(X, affine_transformation):
    from concourse.bass_utils import run_bass_kernel_spmd

    X = np.asarray(X, np.float32)
    theta = np.asarray(affine_transformation, np.float32)
    if "nc" not in _NC_CACHE:
        _NC_CACHE["nc"] = build_nc()
    nc = _NC_CACHE["nc"]
    in_maps = _make_in_maps(X, theta)
    res = run_bass_kernel_spmd(nc, in_maps, list(range(NCORES)))
    outs = [r["OUT"].reshape(BPC, C, H, W) for r in res.results]
    return np.concatenate(outs, axis=0)


if __name__ == "__main__":
    mode = sys.argv[1] if len(sys.argv) > 1 else "sim"
    if mode == "build":
        nc = build_nc()
        print("build ok")
    elif mode == "sim":
        n_chunks = int(sys.argv[2]) if len(sys.argv) > 2 else 2
        import concourse.bass_interp as bass_interp

        _orig_copy = bass_interp.InstructionExecutor.visit_InstTensorCopy

        def _copy_rne(self, instruction, *, reg_snapshot=None):
            from concourse.bass_interp import Direction, InterpAPClass

            inp, outp = instruction.ins[0], instruction.outs[0]
            if isinstance(inp, InterpAPClass) and isinstance(outp, InterpAPClass):
                iv = self.view_ap(
                    inp, Direction.READ, instruction, reg_snapshot=reg_snapshot
                )
                ov = self.view_ap(
                    outp, Direction.WRITE, instruction, reg_snapshot=reg_snapshot
                )
                if np.issubdtype(iv.dtype, np.floating) and np.issubdtype(
                    ov.dtype, np.integer
                ):
                    ov[:] = np.round(iv.reshape(ov.shape))
                    return
            return _orig_copy(self, instruction, reg_snapshot=reg_snapshot)

        bass_interp.InstructionExecutor.visit_InstTensorCopy = _copy_rne

        rng = np.random.default_rng(0)
        Xt = rng.standard_normal((1, C, HW), dtype=np.float32)
        th = rng.standard_normal((1, 6), dtype=np.float32) * 0.7
        nc = build_nc(n_batches=1, n_chunks=n_chunks, debug=False)
        th_rep = np.repeat(th[:, None, :], 128, axis=1)
        sim = bass_interp.CoreSim(nc)
        sim.tensor("X")[:] = Xt
        sim.tensor("THETA")[:] = np.ascontiguousarray(th_rep, np.float32)
        for k, v in _CONSTS.items():
            sim.tensor(k)[:] = v
        sim.simulate()
        got = np.array(sim.tensor("OUT"))

        def ref(Xf, thf):
            xl = np.linspace(-1, 1, W, dtype=np.float32)
            yl = np.linspace(-1, 1, H, dtype=np.float32)
            xc, yc = np.meshgrid(xl, yl, indexing="ij")
            grid = np.stack([xc.ravel(), yc.ravel(), np.ones(W * H, np.float32)], 0)
            thr = thf.reshape(-1, 2, 3)
            sampled = np.einsum("bij,jn->bin", thr, grid)
            x = (sampled[:, 0, :] + 1) * (W * 0.5)
            y = (sampled[:, 1, :] + 1) * (H * 0.5)
            x0 = np.clip(np.floor(x).astype(np.int64), 0, W - 1)
            x1 = np.clip(np.floor(x).astype(np.int64) + 1, 0, W - 1)
            y0 = np.clip(np.floor(y).astype(np.int64), 0, H - 1)
            y1 = np.clip(np.floor(y).astype(np.int64) + 1, 0, H - 1)
            flat = Xf.reshape(-1, C, H * W).transpose(0, 2, 1)
            bidx = np.arange(flat.shape[0])[:, None]
            pa = flat[bidx, y0 * W + x0]
            pb = flat[bidx, y1 * W + x0]
            pc = flat[bidx, y0 * W + x1]
            pd = flat[bidx, y1 * W + x1]
            x0f, x1f, y0f, y1f = (a.astype(np.float32) for a in (x0, x1, y0, y1))
            wa = ((x1f - x) * (y1f - y))[..., None]
            wb = ((x1f - x) * (y - y0f))[..., None]
            wc = ((x - x0f) * (y1f - y))[..., None]
            wd = ((x - x0f) * (y - y0f))[..., None]
            out = wa * pa + wb * pb + wc * pc + wd * pd
            return out.reshape(-1, W, H, C).transpose(0, 3, 2, 1)

        exp_full = ref(Xt, th).reshape(1, C, HW)
        npts = n_chunks * CHUNK_PTS
        got_s = got[0][:, :npts]
        exp_s = exp_full[0][:, :npts]
        err = np.abs(got_s - exp_s)
        denom = np.abs(exp_s).max() + 1e-8
        print("max abs err:", err.max(), " max |exp|:", np.abs(exp_s).max())
        print(
            "rel l2:",
            np.linalg.norm(got_s - exp_s) / (np.linalg.norm(exp_s) + 1e-8),
        )
        bad = np.argwhere(err > 3e-2 * denom)
        print("n bad:", len(bad), "of", got_s.size)
        if len(bad):
            print("first bad:", bad[:5])
